# revision 14
# baseline (speedup 1.0000x reference)
"""LoRA-MoE fused linear (grouped ragged GEMM) on 8 TRN2 NeuronCores.

Strategy: expert-parallel. The 64 experts are LPT-bin-packed onto 8 cores
(8 experts/core) to balance token counts. Tokens are pre-sorted by expert,
so each expert's rows are a contiguous slice of x. The LoRA path is folded
into the base weights on the host (W_eff = w_base + 2 * w_a @ w_b, exact
in fp32 before the bf16 cast), so the device kernel is a pure grouped
GEMM at the bf16 tensor-engine roofline (96 PE-cycles per token column).

Host packs, per core:
  xt : bf16 [2048, CAP]  x^T columns grouped by expert slot (zero padded)
  wb : bf16 [8, 16, 128, 768]   per-slot folded weights, k-chunked
Kernel computes yt[n, c] = sum_k W[k,n] x[c,k] per slot with the token
dim on the matmul free axis, so ragged slot widths need no 128 alignment.
DMAs are issued in few large chunks (SP sequencer costs ~565 ns per
dma_start, so hundreds of small DMAs would throttle the pipeline).
Per-slot capacities are compile-time constants derived from m_sizes
(compiled at call time, cached).
"""

import sys

if "/opt/trn_rl_repo" not in sys.path:
    sys.path.insert(0, "/opt/trn_rl_repo")

import numpy as np
import ml_dtypes

T, IN, OUT, E, R = 32768, 2048, 768, 64, 16
SCALING = 2.0
NC_CORES = 8
EPC = E // NC_CORES  # experts per core
KC = IN // 128       # 16 contraction chunks
NT = OUT // 128      # 6 output-feature tiles
MAX_N = 512          # PSUM bank limit (fp32 columns)
BF16 = ml_dtypes.bfloat16

# kc-chunk grouping for input DMAs: first chunk small so the first matmul's
# operands land early, rest large to keep the dma_start count low.
KC_GROUPS = ((0, 1), (1, 4), (4, 10), (10, 16))

_cache: dict = {}


def _split_sync_waits(nc, max_waits=1):
    """walrus in this container rejects >1 sync-wait on an instruction;
    split extras onto preceding NoOps on the same engine."""
    import concourse.mybir as mybir

    n_split = 0
    for fn in nc.m.functions:
        for bb in fn.blocks:
            new_insts = []
            for ins in bb.instructions:
                si = getattr(ins, "sync_info", None)
                waits = list(si.on_wait) if si is not None and si.on_wait else []
                if len(waits) > max_waits:
                    k = 0
                    while len(waits) - k > max_waits:
                        chunk = waits[k : k + max_waits]
                        k += max_waits
                        nop = mybir.InstNoOp(
                            name=nc.get_next_instruction_name(),
                            ins=[],
                            outs=[],
                            sync_info=mybir.SyncInfo(on_wait=chunk, on_update=[]),
                        )
                        nop.engine = ins.engine
                        new_insts.append(nop)
                        n_split += 1
                    si.on_wait = waits[k:]
                new_insts.append(ins)
            bb.instructions[:] = new_insts
    return n_split


def _mtiles(cap):
    """Split a slot's column span into even tiles of <= MAX_N."""
    nt = -(-cap // MAX_N)
    base = -(-(-(-cap // nt)) // 4) * 4  # ceil(cap/nt) rounded up to mult of 4
    tiles = []
    c0 = 0
    for i in range(nt):
        ml = min(base, cap - c0)
        if ml <= 0:
            break
        tiles.append((c0, ml))
        c0 += ml
    return tiles


def _build(caps, rep=1, loop_n=1):
    """Build the kernel; `rep` unrolls the body, `loop_n` > 1 additionally
    wraps it in a hardware For_i loop (timing amplification at constant
    program size)."""
    import contextlib

    import concourse.bass as bass
    import concourse.mybir as mybir
    import concourse.tile as tile

    CAP = int(sum(caps))
    nc = bass.Bass()
    xt_h = nc.declare_dram_parameter("xt", [IN, CAP], mybir.dt.bfloat16, isOutput=False)
    wb_h = nc.declare_dram_parameter(
        "wb", [EPC, KC, 128, OUT], mybir.dt.bfloat16, isOutput=False
    )
    yt_h = nc.declare_dram_parameter("yt", [OUT, CAP], mybir.dt.bfloat16, isOutput=True)

    xt = xt_h[:].rearrange("(kc kp) c -> kp kc c", kp=128)  # [128, KC, CAP]
    yt = yt_h[:].rearrange("(nt np) c -> np nt c", np=128)  # [128, NT, CAP]

    with tile.TileContext(nc) as tc:
        with (
            tc.tile_pool(name="xtp", bufs=4) as xtp,
            tc.tile_pool(name="wbp", bufs=4) as wbp,
            tc.tile_pool(name="outp", bufs=3) as outp,
            tc.tile_pool(name="psf", bufs=1, space="PSUM") as psf,
            tc.tile_pool(name="psp", bufs=2, space="PSUM") as psp,
        ):
          with (tc.For_i(0, loop_n) if loop_n > 1 else contextlib.nullcontext()):
            for _rep in range(rep):
                col0 = 0
                for s, cap in enumerate(caps):
                    cap = int(cap)
                    if cap == 0:
                        continue
                    mtiles = _mtiles(cap)
                    xts = xtp.tile([128, KC, cap], mybir.dt.bfloat16, tag="xts")
                    wbs = wbp.tile([128, KC, OUT], mybir.dt.bfloat16, tag="wbs")
                    wb_src = wb_h[s].rearrange("kc kp n -> kp kc n")
                    for k0, k1 in KC_GROUPS:
                        # two HWDGE queues: x via SP, weights via Activation,
                        # so the two input streams transfer in parallel
                        if s == 0 and len(mtiles) > 1:
                            # column-split so the fill-phase mtile's matmuls
                            # depend only on its own columns
                            ml0 = mtiles[0][1]
                            nc.sync.dma_start(
                                out=xts[:, k0:k1, :ml0],
                                in_=xt[:, k0:k1, col0 : col0 + ml0],
                            )
                            nc.sync.dma_start(
                                out=xts[:, k0:k1, ml0:],
                                in_=xt[:, k0:k1, col0 + ml0 : col0 + cap],
                            )
                        else:
                            nc.sync.dma_start(
                                out=xts[:, k0:k1, :],
                                in_=xt[:, k0:k1, col0 : col0 + cap],
                            )
                        nc.scalar.dma_start(
                            out=wbs[:, k0:k1, :], in_=wb_src[:, k0:k1, :]
                        )

                    for mi, (c0, ml) in enumerate(mtiles):
                        outs = outp.tile([128, NT, ml], mybir.dt.bfloat16, tag="outs")
                        if s == 0 and mi == 0:
                            # pipeline-fill mtile: kc-outer with all NT psum
                            # groups open, consuming each k-chunk as its DMA
                            # lands rather than waiting for the whole slot
                            pss = [
                                psf.tile(
                                    [128, ml],
                                    mybir.dt.float32,
                                    tag=f"psf{nt}",
                                    name=f"psf{nt}",
                                )
                                for nt in range(NT)
                            ]
                            for kc in range(KC):
                                for nt in range(NT):
                                    nc.tensor.matmul(
                                        pss[nt][:],
                                        wbs[:, kc, nt * 128 : (nt + 1) * 128],
                                        xts[:, kc, c0 : c0 + ml],
                                        start=(kc == 0),
                                        stop=(kc == KC - 1),
                                    )
                            for nt in range(NT):
                                nc.vector.tensor_copy(outs[:, nt, :], pss[nt][:])
                                nc.sync.dma_start(
                                    out=yt[:, nt, col0 + c0 : col0 + c0 + ml],
                                    in_=outs[:, nt, :],
                                )
                            continue
                        for nt in range(NT):
                            ps = psp.tile([128, ml], mybir.dt.float32, tag="ps")
                            for kc in range(KC):
                                nc.tensor.matmul(
                                    ps[:],
                                    wbs[:, kc, nt * 128 : (nt + 1) * 128],
                                    xts[:, kc, c0 : c0 + ml],
                                    start=(kc == 0),
                                    stop=(kc == KC - 1),
                                )
                            nc.vector.tensor_copy(outs[:, nt, :], ps[:])
                            # per-NT output DMA overlaps the remaining groups
                            # and shrinks the end-of-kernel drain
                            nc.sync.dma_start(
                                out=yt[:, nt, col0 + c0 : col0 + c0 + ml],
                                in_=outs[:, nt, :],
                            )
                    col0 += cap

    _split_sync_waits(nc)
    return nc


def _plan(m_sizes):
    """LPT-balanced assignment of experts to cores; per-slot capacities."""
    m = np.asarray(m_sizes, dtype=np.int64)
    offs = np.zeros(E + 1, dtype=np.int64)
    np.cumsum(np.maximum(m, 0), out=offs[1:])
    # effective sizes clipped to the token count
    starts = np.minimum(offs[:-1], T)
    ends = np.minimum(offs[1:], T)
    eff = ends - starts

    order = np.argsort(-eff, kind="stable")
    load = np.zeros(NC_CORES, dtype=np.int64)
    slots = [[] for _ in range(NC_CORES)]
    for e in order:
        cands = [c for c in range(NC_CORES) if len(slots[c]) < EPC]
        c = min(cands, key=lambda i: (load[i], i))
        slots[c].append(int(e))
        load[c] += eff[e]
    # slots[c] is descending in eff by construction
    caps = tuple(
        int(-(-max(int(eff[slots[c][s]]) for c in range(NC_CORES)) // 4) * 4)
        for s in range(EPC)
    )
    return slots, caps, starts, eff


def plan_caps(m_sizes):
    return _plan(m_sizes)[1]


def prepare_in_maps(x, m_sizes, w_base, w_a, w_b):
    x = np.ascontiguousarray(np.asarray(x), dtype=np.float32)
    w_base = np.asarray(w_base, dtype=np.float32)
    w_a = np.asarray(w_a, dtype=np.float32)
    w_b = np.asarray(w_b, dtype=np.float32)

    slots, caps, starts, eff = _plan(m_sizes)
    CAP = int(sum(caps))
    colstart = np.zeros(EPC + 1, dtype=np.int64)
    np.cumsum(np.asarray(caps), out=colstart[1:])

    xb = x.astype(BF16)
    # Fold the LoRA path into the base weights (exact in fp32).
    weff = w_base + SCALING * np.matmul(w_a, w_b)
    wbb = weff.astype(BF16)

    in_maps = []
    for c in range(NC_CORES):
        exps = slots[c]
        xt = np.zeros((IN, CAP), dtype=BF16)
        for s, e in enumerate(exps):
            n = int(eff[e])
            if n:
                xt[:, colstart[s] : colstart[s] + n] = xb[
                    starts[e] : starts[e] + n
                ].T
        in_maps.append(
            {
                "xt": xt,
                "wb": np.ascontiguousarray(wbb[exps]).reshape(EPC, KC, 128, OUT),
            }
        )
    return in_maps


def kernel(x, m_sizes, w_base, w_a, w_b):
    slots, caps, starts, eff = _plan(m_sizes)
    key = caps
    if key not in _cache:
        _cache[key] = _build(caps)
    nc = _cache[key]

    colstart = np.zeros(EPC + 1, dtype=np.int64)
    np.cumsum(np.asarray(caps), out=colstart[1:])

    in_maps = prepare_in_maps(x, m_sizes, w_base, w_a, w_b)

    from concourse.bass_utils import run_bass_kernel_spmd

    res = run_bass_kernel_spmd(nc, in_maps, core_ids=list(range(NC_CORES)))

    out = np.zeros((T, OUT), dtype=np.float32)
    for c in range(NC_CORES):
        yt = res.results[c]["yt"]
        for s, e in enumerate(slots[c]):
            n = int(eff[e])
            if n:
                out[starts[e] : starts[e] + n] = (
                    yt[:, colstart[s] : colstart[s] + n].T.astype(np.float32)
                )
    return out


# revision 16
# speedup vs baseline: 1.2911x; 1.2911x over previous
"""LoRA-MoE fused linear (grouped ragged GEMM) on 8 TRN2 NeuronCores.

Strategy: expert-parallel. The 64 experts are LPT-bin-packed onto 8 cores
(8 experts/core) to balance token counts. Tokens are pre-sorted by expert,
so each expert's rows are a contiguous slice of x. The LoRA path is folded
into the base weights on the host (W_eff = w_base + 2 * w_a @ w_b, exact
in fp32 before the bf16 cast), so the device kernel is a pure grouped
GEMM at the bf16 tensor-engine roofline (96 PE-cycles per token column).

Host packs, per core:
  xt : bf16 [2048, CAP]  x^T columns grouped by expert slot (zero padded)
  wb : bf16 [8, 16, 128, 768]   per-slot folded weights, k-chunked
Kernel computes yt[n, c] = sum_k W[k,n] x[c,k] per slot with the token
dim on the matmul free axis, so ragged slot widths need no 128 alignment.
DMAs are issued in few large chunks (SP sequencer costs ~565 ns per
dma_start, so hundreds of small DMAs would throttle the pipeline).
Per-slot capacities are compile-time constants derived from m_sizes
(compiled at call time, cached).
"""

import sys

if "/opt/trn_rl_repo" not in sys.path:
    sys.path.insert(0, "/opt/trn_rl_repo")

import numpy as np
import ml_dtypes

T, IN, OUT, E, R = 32768, 2048, 768, 64, 16
SCALING = 2.0
NC_CORES = 8
EPC = E // NC_CORES  # experts per core
KC = IN // 128       # 16 contraction chunks
NT = OUT // 128      # 6 output-feature tiles
MAX_N = 512          # PSUM bank limit (fp32 columns)
BF16 = ml_dtypes.bfloat16

# kc-chunk grouping for input DMAs: first chunk small so the first matmul's
# operands land early, rest large to keep the dma_start count low.
KC_GROUPS = ((0, 1), (1, 4), (4, 10), (10, 16))

_cache: dict = {}


def _split_sync_waits(nc, max_waits=1):
    """walrus in this container rejects >1 sync-wait on an instruction;
    split extras onto preceding NoOps on the same engine."""
    import concourse.mybir as mybir

    n_split = 0
    for fn in nc.m.functions:
        for bb in fn.blocks:
            new_insts = []
            for ins in bb.instructions:
                si = getattr(ins, "sync_info", None)
                waits = list(si.on_wait) if si is not None and si.on_wait else []
                if len(waits) > max_waits:
                    k = 0
                    while len(waits) - k > max_waits:
                        chunk = waits[k : k + max_waits]
                        k += max_waits
                        nop = mybir.InstNoOp(
                            name=nc.get_next_instruction_name(),
                            ins=[],
                            outs=[],
                            sync_info=mybir.SyncInfo(on_wait=chunk, on_update=[]),
                        )
                        nop.engine = ins.engine
                        new_insts.append(nop)
                        n_split += 1
                    si.on_wait = waits[k:]
                new_insts.append(ins)
            bb.instructions[:] = new_insts
    return n_split


def _mtiles(cap):
    """Split a slot's column span into even tiles of <= MAX_N."""
    nt = -(-cap // MAX_N)
    base = -(-(-(-cap // nt)) // 4) * 4  # ceil(cap/nt) rounded up to mult of 4
    tiles = []
    c0 = 0
    for i in range(nt):
        ml = min(base, cap - c0)
        if ml <= 0:
            break
        tiles.append((c0, ml))
        c0 += ml
    return tiles


def _build(caps, rep=1, loop_n=1):
    """Build the kernel; `rep` unrolls the body, `loop_n` > 1 additionally
    wraps it in a hardware For_i loop (timing amplification at constant
    program size)."""
    import contextlib

    import concourse.bass as bass
    import concourse.mybir as mybir
    import concourse.tile as tile

    CAP = int(sum(caps))
    nc = bass.Bass()
    xt_h = nc.declare_dram_parameter("xt", [IN, CAP], mybir.dt.bfloat16, isOutput=False)
    wb_h = nc.declare_dram_parameter(
        "wb", [EPC, KC, 128, OUT], mybir.dt.bfloat16, isOutput=False
    )
    yt_h = nc.declare_dram_parameter("yt", [OUT, CAP], mybir.dt.bfloat16, isOutput=True)

    xt = xt_h[:].rearrange("(kc kp) c -> kp kc c", kp=128)  # [128, KC, CAP]
    yt = yt_h[:].rearrange("(nt np) c -> np nt c", np=128)  # [128, NT, CAP]

    with tile.TileContext(nc) as tc:
        with (
            tc.tile_pool(name="xtp", bufs=4) as xtp,
            tc.tile_pool(name="wbp", bufs=4) as wbp,
            tc.tile_pool(name="outp", bufs=3) as outp,
            tc.tile_pool(name="psp", bufs=6, space="PSUM") as psp,
        ):
          with (tc.For_i(0, loop_n) if loop_n > 1 else contextlib.nullcontext()):
            for _rep in range(rep):
                col0 = 0
                for s, cap in enumerate(caps):
                    cap = int(cap)
                    if cap == 0:
                        continue
                    xts = xtp.tile([128, KC, cap], mybir.dt.bfloat16, tag="xts")
                    wbs = wbp.tile([128, KC, OUT], mybir.dt.bfloat16, tag="wbs")
                    wb_src = wb_h[s].rearrange("kc kp n -> kp kc n")
                    for k0, k1 in KC_GROUPS:
                        # two HWDGE queues: x via SP, weights via Activation,
                        # so the two input streams transfer in parallel
                        nc.sync.dma_start(
                            out=xts[:, k0:k1, :],
                            in_=xt[:, k0:k1, col0 : col0 + cap],
                        )
                        nc.scalar.dma_start(
                            out=wbs[:, k0:k1, :], in_=wb_src[:, k0:k1, :]
                        )

                    for c0, ml in _mtiles(cap):
                        outs = outp.tile([128, NT, ml], mybir.dt.bfloat16, tag="outs")
                        for nt in range(NT):
                            ps = psp.tile([128, ml], mybir.dt.float32, tag="ps")
                            for kc in range(KC):
                                nc.tensor.matmul(
                                    ps[:],
                                    wbs[:, kc, nt * 128 : (nt + 1) * 128],
                                    xts[:, kc, c0 : c0 + ml],
                                    start=(kc == 0),
                                    stop=(kc == KC - 1),
                                )
                            nc.vector.tensor_copy(outs[:, nt, :], ps[:])
                            # per-NT output DMA overlaps the remaining groups
                            # and shrinks the end-of-kernel drain
                            nc.sync.dma_start(
                                out=yt[:, nt, col0 + c0 : col0 + c0 + ml],
                                in_=outs[:, nt, :],
                            )
                    col0 += cap

    _split_sync_waits(nc)
    return nc


def _plan(m_sizes):
    """LPT-balanced assignment of experts to cores; per-slot capacities."""
    m = np.asarray(m_sizes, dtype=np.int64)
    offs = np.zeros(E + 1, dtype=np.int64)
    np.cumsum(np.maximum(m, 0), out=offs[1:])
    # effective sizes clipped to the token count
    starts = np.minimum(offs[:-1], T)
    ends = np.minimum(offs[1:], T)
    eff = ends - starts

    order = np.argsort(-eff, kind="stable")
    load = np.zeros(NC_CORES, dtype=np.int64)
    slots = [[] for _ in range(NC_CORES)]
    for e in order:
        cands = [c for c in range(NC_CORES) if len(slots[c]) < EPC]
        c = min(cands, key=lambda i: (load[i], i))
        slots[c].append(int(e))
        load[c] += eff[e]
    # slots[c] is descending in eff by construction
    caps = tuple(
        int(-(-max(int(eff[slots[c][s]]) for c in range(NC_CORES)) // 4) * 4)
        for s in range(EPC)
    )
    return slots, caps, starts, eff


def plan_caps(m_sizes):
    return _plan(m_sizes)[1]


def prepare_in_maps(x, m_sizes, w_base, w_a, w_b):
    x = np.ascontiguousarray(np.asarray(x), dtype=np.float32)
    w_base = np.asarray(w_base, dtype=np.float32)
    w_a = np.asarray(w_a, dtype=np.float32)
    w_b = np.asarray(w_b, dtype=np.float32)

    slots, caps, starts, eff = _plan(m_sizes)
    CAP = int(sum(caps))
    colstart = np.zeros(EPC + 1, dtype=np.int64)
    np.cumsum(np.asarray(caps), out=colstart[1:])

    xb = x.astype(BF16)
    # Fold the LoRA path into the base weights (exact in fp32).
    weff = w_base + SCALING * np.matmul(w_a, w_b)
    wbb = weff.astype(BF16)

    in_maps = []
    for c in range(NC_CORES):
        exps = slots[c]
        xt = np.zeros((IN, CAP), dtype=BF16)
        for s, e in enumerate(exps):
            n = int(eff[e])
            if n:
                xt[:, colstart[s] : colstart[s] + n] = xb[
                    starts[e] : starts[e] + n
                ].T
        in_maps.append(
            {
                "xt": xt,
                "wb": np.ascontiguousarray(wbb[exps]).reshape(EPC, KC, 128, OUT),
            }
        )
    return in_maps


def kernel(x, m_sizes, w_base, w_a, w_b):
    slots, caps, starts, eff = _plan(m_sizes)
    key = caps
    if key not in _cache:
        _cache[key] = _build(caps)
    nc = _cache[key]

    colstart = np.zeros(EPC + 1, dtype=np.int64)
    np.cumsum(np.asarray(caps), out=colstart[1:])

    in_maps = prepare_in_maps(x, m_sizes, w_base, w_a, w_b)

    from concourse.bass_utils import run_bass_kernel_spmd

    res = run_bass_kernel_spmd(nc, in_maps, core_ids=list(range(NC_CORES)))

    out = np.zeros((T, OUT), dtype=np.float32)
    for c in range(NC_CORES):
        yt = res.results[c]["yt"]
        for s, e in enumerate(slots[c]):
            n = int(eff[e])
            if n:
                out[starts[e] : starts[e] + n] = (
                    yt[:, colstart[s] : colstart[s] + n].T.astype(np.float32)
                )
    return out
